# revision 7
# baseline (speedup 1.0000x reference)
"""Talking-heads attention with GFSA reaction term — TRN2 Bass kernel, 8 cores.

Sharding: (batch b, query-half) -> core c = b*2 + half. Each core handles all 12
heads for its 512 query rows (m/key axis kept full). The only cross-core data
dependency is w1 = attn3 @ v (needed over the FULL query axis by the second
reaction matmul), exchanged with a tiny per-pair AllGather (1.5MB).

Math (associativity rewrite — never materialize attn3 @ attn3):
  attn1[h]  = (q_h * SCALE) @ k_h^T
  attn2[g]  = sum_h W1[g,h] attn1[h] + b1[g]          (folded into QK^T matmuls)
  E_g       = exp(attn2[g]),  Z_g[n] = sum_m E_g      (fp16, [m,n] layout)
  attn3[g'] = sum_g W2[g',g] E_g/Z_g + b2[g']          (folded into A@v passes)
  w1[g']    = attn3[g'] @ v_g'
  out[g']   = attn3[g'] @ ((1-2*lam)v_g' + 3*lam*w1[g'])
  y         = out @ Wo^T + ob                          (n-local, concat on host)

Matmul dtypes: fp32r (full-rate fp32) for QKV/outproj; fp16 for scores and the
probability-matrix passes (E stored fp16 in SBUF, fp32 PSUM accumulation).
"""
import numpy as np

import concourse.bacc as bacc
import concourse.mybir as mybir
import concourse.tile as tile
from concourse.bass_utils import run_bass_kernel_spmd
from concourse.masks import make_identity

B, N, DIM, HEADS, HD = 4, 1024, 768, 12, 64
NH = N // 2                      # query rows per core
SCALE = HD ** -0.5
f32 = mybir.dt.float32
f32r = mybir.dt.float32r
f16 = mybir.dt.float16
AL = mybir.AluOpType
AF = mybir.ActivationFunctionType

TRACE = False                    # test.py may flip this for profiling
TRACE_KW = {}
DEBUG = False                    # dump intermediates as extra outputs


def _build():
    nc = bacc.Bacc("TRN2", target_bir_lowering=False, debug=False, num_devices=8)

    xh_T = nc.declare_dram_parameter("xh_T", [DIM, NH], f32r, isOutput=False)
    xf_T = nc.declare_dram_parameter("xf_T", [DIM, N], f32r, isOutput=False)
    wq_T = nc.declare_dram_parameter("wq_T", [DIM, DIM], f32r, isOutput=False)
    wk_T = nc.declare_dram_parameter("wk_T", [DIM, DIM], f32r, isOutput=False)
    wv_T = nc.declare_dram_parameter("wv_T", [DIM, DIM], f32r, isOutput=False)
    wo_T = nc.declare_dram_parameter("wo_T", [DIM, DIM], f32r, isOutput=False)
    w1v = nc.declare_dram_parameter("w1v", [128, 72], f32, isOutput=False)
    b1bc = nc.declare_dram_parameter("b1bc", [128, HEADS], f32, isOutput=False)
    w2f = nc.declare_dram_parameter("w2f", [1, HEADS * DIM], f32, isOutput=False)
    uc1 = nc.declare_dram_parameter("uc1", [1, DIM], f32, isOutput=False)
    uc2 = nc.declare_dram_parameter("uc2", [1, DIM], f32, isOutput=False)
    b2blk = nc.declare_dram_parameter("b2blk", [1, DIM], f32, isOutput=False)
    ob = nc.declare_dram_parameter("ob", [1, DIM], f32, isOutput=False)
    y = nc.declare_dram_parameter("y", [NH, DIM], f32, isOutput=True)
    if DEBUG:
        dbg_q = nc.declare_dram_parameter("dbg_q", [128, 6, NH], mybir.dt.float16, isOutput=True)
        dbg_k = nc.declare_dram_parameter("dbg_k", [128, 6, N], mybir.dt.float16, isOutput=True)
        dbg_v = nc.declare_dram_parameter("dbg_v", [128, 8, DIM], mybir.dt.float16, isOutput=True)
        dbg_E = nc.declare_dram_parameter("dbg_E", [128, 8, HEADS, NH], mybir.dt.float16, isOutput=True)
        dbg_zr = nc.declare_dram_parameter("dbg_zr", [128, HEADS, 4], f32, isOutput=True)
        dbg_w1 = nc.declare_dram_parameter("dbg_w1", [128, 4, DIM], f32, isOutput=True)
        dbg_w1f = nc.declare_dram_parameter("dbg_w1f", [128, 8, DIM], mybir.dt.float16, isOutput=True)
        dbg_u = nc.declare_dram_parameter("dbg_u", [128, 8, DIM], mybir.dt.float16, isOutput=True)
        dbg_acc2 = nc.declare_dram_parameter("dbg_acc2", [128, 4, DIM], f32, isOutput=True)

    with tile.TileContext(nc) as tc:
        with tc.tile_pool(name="persist", bufs=1) as pp:
            # [m%128, m//128, g, n_local] fp16 exp'd mixed scores (transposed layout)
            E = pp.tile([128, 8, HEADS, NH], f16)
            v16 = pp.tile([128, 8, DIM], f16)          # [m%128, m//128, (g',d)]
            Zr = pp.tile([128, HEADS, 4], f32)         # 1/Z per (g, n-subtile)
            w1v_sb = pp.tile([128, 72], f32)
            b1_sb = pp.tile([128, HEADS], f32)
            uc1bc = pp.tile([128, DIM], f32)
            uc2bc = pp.tile([128, DIM], f32)
            b2bc = pp.tile([128, DIM], f32)
            ones128 = pp.tile([128, 128], f16)
            nc.sync.dma_start(w1v_sb[:], w1v[:])
            nc.sync.dma_start(b1_sb[:], b1bc[:])
            nc.sync.dma_start(uc1bc[:], uc1[0:1, :].to_broadcast((128, DIM)))
            nc.sync.dma_start(uc2bc[:], uc2[0:1, :].to_broadcast((128, DIM)))
            nc.sync.dma_start(b2bc[:], b2blk[0:1, :].to_broadcast((128, DIM)))
            nc.vector.memset(ones128[:], 1.0)

            with tc.tile_pool(name="qk", bufs=1) as qk:
                qT = qk.tile([128, 6, NH], f16)        # [d%128, d//128, n_local]
                kT = qk.tile([128, 6, N], f16)         # [d%128, d//128, m]

                # ---- Phase A: QKV projections -------------------------------
                with tc.tile_pool(name="pha", bufs=1) as pa, \
                     tc.tile_pool(name="wsl", bufs=4) as wsl, \
                     tc.tile_pool(name="wvr", bufs=2) as wvr, \
                     tc.tile_pool(name="psa", bufs=2, space="PSUM") as psa:
                    xf = pa.tile([128, 6, N], f32r)
                    xh = pa.tile([128, 6, NH], f32r)
                    nc.sync.dma_start(xf[:], xf_T.rearrange("(c p) n -> p c n", p=128))
                    nc.sync.dma_start(xh[:], xh_T.rearrange("(c p) n -> p c n", p=128))
                    wqr = wq_T.rearrange("(c p) n -> p c n", p=128)
                    wkr = wk_T.rearrange("(c p) n -> p c n", p=128)
                    wvr_ap = wv_T.rearrange("(c p) n -> p c n", p=128)

                    for qc in range(6):                # qT[c, n] = sum_d wq[d,c]x[n,d]
                        ps = psa.tile([128, NH], f32, tag="ps512")
                        for d in range(6):
                            wsl_t = wsl.tile([128, 128], f32r, tag="w")
                            nc.sync.dma_start(wsl_t[:], wqr[:, d, qc * 128:(qc + 1) * 128])
                            nc.tensor.matmul(ps[:], wsl_t[:], xh[:, d, :],
                                             start=(d == 0), stop=(d == 5))
                        nc.vector.tensor_copy(qT[:, qc, :], ps[:])
                    for kc in range(6):
                        for mc in range(2):
                            ps = psa.tile([128, 512], f32, tag="ps512")
                            for d in range(6):
                                wsl_t = wsl.tile([128, 128], f32r, tag="w")
                                nc.sync.dma_start(wsl_t[:], wkr[:, d, kc * 128:(kc + 1) * 128])
                                nc.tensor.matmul(ps[:], wsl_t[:],
                                                 xf[:, d, mc * 512:(mc + 1) * 512],
                                                 start=(d == 0), stop=(d == 5))
                            nc.vector.tensor_copy(kT[:, kc, mc * 512:(mc + 1) * 512], ps[:])
                    for vc in range(2):                # v[m, c] = sum_d x[m,d]wv[d,c]
                        wv_t = wvr.tile([128, 6, 384], f32r, tag="wv")
                        for d in range(6):
                            nc.sync.dma_start(wv_t[:, d, :],
                                              wvr_ap[:, d, vc * 384:(vc + 1) * 384])
                        for mt in range(8):
                            ps = psa.tile([128, 384], f32, tag="ps384")
                            for d in range(6):
                                nc.tensor.matmul(ps[:], xf[:, d, mt * 128:(mt + 1) * 128],
                                                 wv_t[:, d, :],
                                                 start=(d == 0), stop=(d == 5))
                            nc.vector.tensor_copy(v16[:, mt, vc * 384:(vc + 1) * 384], ps[:])

                # ---- Phase B: mixed scores (mix1 fold) + exp ----------------
                with tc.tile_pool(name="qsc", bufs=2) as qscp, \
                     tc.tile_pool(name="psb", bufs=4, space="PSUM") as psb:
                    for g in range(HEADS):
                        qsc = qscp.tile([128, 6, NH], f16, tag="qsc")
                        for i in range(6):
                            nc.scalar.activation(qsc[:, i, :], qT[:, i, :], AF.Copy,
                                                 scale=w1v_sb[:, g * 6 + i:g * 6 + i + 1])
                        for mt in range(8):
                            ps = psb.tile([128, NH], f32, tag="psb")
                            for i in range(6):
                                nc.tensor.matmul(ps[:], kT[:, i, mt * 128:(mt + 1) * 128],
                                                 qsc[:, i, :], start=(i == 0), stop=(i == 5))
                            nc.scalar.activation(E[:, mt, g, :], ps[:], AF.Exp,
                                                 bias=b1_sb[:, g:g + 1], scale=1.0)

                if DEBUG:
                    nc.sync.dma_start(dbg_q[:], qT[:])
                    nc.sync.dma_start(dbg_k[:], kT[:])

            if DEBUG:
                nc.sync.dma_start(dbg_v[:], v16[:])
                nc.sync.dma_start(dbg_E[:], E[:])

            with tc.tile_pool(name="late", bufs=1) as late:
                w1f = late.tile([128, 8, DIM], f16)
                acc2 = late.tile([128, 4, DIM], f32)
                u16 = late.tile([128, 8, DIM], f16)

                # ---- Phase C: pass-1  w1 = attn3 @ v  (mix2 + 1/Z folded) ---
                with tc.tile_pool(name="phc", bufs=1) as pc, \
                     tc.tile_pool(name="vt", bufs=1) as vtp, \
                     tc.tile_pool(name="w2p", bufs=2) as w2p, \
                     tc.tile_pool(name="tmpc", bufs=2) as tmpp, \
                     tc.tile_pool(name="psc", bufs=2, space="PSUM") as psc, \
                     tc.tile_pool(name="pscv", bufs=1, space="PSUM") as pscv:
                    w1acc = pc.tile([128, 4, DIM], f32)
                    nc.vector.memset(w1acc[:], 0.0)
                    for g in range(HEADS):
                        w2row = w2p.tile([128, DIM], f32, tag="w2row")
                        nc.sync.dma_start(
                            w2row[:],
                            w2f[0:1, g * DIM:(g + 1) * DIM].to_broadcast((128, DIM)))
                        Vt = vtp.tile([128, 8, 769], f16, tag="vt")
                        for mt in range(8):
                            nc.vector.tensor_tensor(Vt[:, mt, 0:DIM], v16[:, mt, :],
                                                    w2row[:], AL.mult)
                        nc.vector.memset(Vt[:, :, 768], 1.0)
                        for ns in range(4):
                            psA = psc.tile([128, 512], f32, tag="pscA")
                            psB = psc.tile([128, 257], f32, tag="pscB")
                            for mt in range(8):
                                lhs = E[:, mt, g, ns * 128:(ns + 1) * 128]
                                nc.tensor.matmul(psA[:], lhs, Vt[:, mt, 0:512],
                                                 start=(mt == 0), stop=(mt == 7))
                                nc.tensor.matmul(psB[:], lhs, Vt[:, mt, 512:769],
                                                 start=(mt == 0), stop=(mt == 7))
                            zr = Zr[:, g, ns:ns + 1]
                            nc.vector.reciprocal(zr, psB[:, 256:257])
                            tA = tmpp.tile([128, 512], f32, tag="tA")
                            tB = tmpp.tile([128, 256], f32, tag="tB")
                            nc.scalar.activation(tA[:], psA[:], AF.Copy, scale=zr)
                            nc.scalar.activation(tB[:], psB[:, 0:256], AF.Copy, scale=zr)
                            nc.vector.tensor_add(w1acc[:, ns, 0:512],
                                                 w1acc[:, ns, 0:512], tA[:])
                            nc.vector.tensor_add(w1acc[:, ns, 512:768],
                                                 w1acc[:, ns, 512:768], tB[:])
                    # + b2[g'] * colsum(v)   (ones128 lhsT -> row-replicated sums)
                    psV = pscv.tile([128, DIM], f32)
                    for mt in range(8):
                        nc.tensor.matmul(psV[:, 0:512], ones128[:], v16[:, mt, 0:512],
                                         start=(mt == 0), stop=(mt == 7))
                    for mt in range(8):
                        nc.tensor.matmul(psV[:, 512:768], ones128[:], v16[:, mt, 512:768],
                                         start=(mt == 0), stop=(mt == 7))
                    b2v = pc.tile([128, DIM], f32)
                    nc.vector.tensor_tensor(b2v[:], psV[:], b2bc[:], AL.mult)
                    for ns in range(4):
                        nc.vector.tensor_add(w1acc[:, ns, :], w1acc[:, ns, :], b2v[:])

                    if DEBUG:
                        nc.sync.dma_start(dbg_zr[:], Zr[:])
                        nc.sync.dma_start(dbg_w1[:], w1acc[:])

                    # ---- Phase D: AllGather w1 across the batch pair --------
                    with tc.tile_pool(name="dram", bufs=1, space="DRAM") as dram:
                        w1loc = dram.tile([NH, DIM], f32)
                        w1full = dram.tile([N, DIM], f32)
                        nc.sync.dma_start(
                            w1loc.rearrange("(ns p) j -> p ns j", p=128), w1acc[:])
                        nc.gpsimd.collective_compute(
                            "AllGather", AL.bypass,
                            replica_groups=[[0, 1], [2, 3], [4, 5], [6, 7]],
                            ins=[w1loc.opt()], outs=[w1full.opt()])
                        nc.gpsimd.dma_start(
                            w1f[:], w1full.rearrange("(mt p) j -> p mt j", p=128))

                # ---- Phase E: pass-2  out = attn3 @ u,  u=(1-2L)v+3L*w1 -----
                with tc.tile_pool(name="vt2", bufs=1) as vtp2, \
                     tc.tile_pool(name="w2p2", bufs=2) as w2p2, \
                     tc.tile_pool(name="tmpe", bufs=2) as tmpe, \
                     tc.tile_pool(name="pse", bufs=2, space="PSUM") as pse, \
                     tc.tile_pool(name="psev", bufs=1, space="PSUM") as psev:
                    nc.vector.memset(acc2[:], 0.0)
                    for mt in range(8):
                        t1 = tmpe.tile([128, DIM], f16, tag="t1")
                        t2 = tmpe.tile([128, DIM], f16, tag="t2")
                        nc.vector.tensor_tensor(t1[:], v16[:, mt, :], uc1bc[:], AL.mult)
                        nc.vector.tensor_tensor(t2[:], w1f[:, mt, :], uc2bc[:], AL.mult)
                        nc.vector.tensor_add(u16[:, mt, :], t1[:], t2[:])
                    for g in range(HEADS):
                        w2row = w2p2.tile([128, DIM], f32, tag="w2row2")
                        nc.sync.dma_start(
                            w2row[:],
                            w2f[0:1, g * DIM:(g + 1) * DIM].to_broadcast((128, DIM)))
                        Ut = vtp2.tile([128, 8, DIM], f16, tag="ut")
                        for mt in range(8):
                            nc.vector.tensor_tensor(Ut[:, mt, :], u16[:, mt, :],
                                                    w2row[:], AL.mult)
                        for ns in range(4):
                            psA = pse.tile([128, 512], f32, tag="pseA")
                            psB = pse.tile([128, 256], f32, tag="pseB")
                            for mt in range(8):
                                lhs = E[:, mt, g, ns * 128:(ns + 1) * 128]
                                nc.tensor.matmul(psA[:], lhs, Ut[:, mt, 0:512],
                                                 start=(mt == 0), stop=(mt == 7))
                                nc.tensor.matmul(psB[:], lhs, Ut[:, mt, 512:768],
                                                 start=(mt == 0), stop=(mt == 7))
                            zr = Zr[:, g, ns:ns + 1]
                            tA = tmpe.tile([128, 512], f32, tag="teA")
                            tB = tmpe.tile([128, 256], f32, tag="teB")
                            nc.scalar.activation(tA[:], psA[:], AF.Copy, scale=zr)
                            nc.scalar.activation(tB[:], psB[:], AF.Copy, scale=zr)
                            nc.vector.tensor_add(acc2[:, ns, 0:512],
                                                 acc2[:, ns, 0:512], tA[:])
                            nc.vector.tensor_add(acc2[:, ns, 512:768],
                                                 acc2[:, ns, 512:768], tB[:])
                    # + b2[g'] * colsum(u)
                    psU = psev.tile([128, DIM], f32)
                    for mt in range(8):
                        nc.tensor.matmul(psU[:, 0:512], ones128[:], u16[:, mt, 0:512],
                                         start=(mt == 0), stop=(mt == 7))
                    for mt in range(8):
                        nc.tensor.matmul(psU[:, 512:768], ones128[:], u16[:, mt, 512:768],
                                         start=(mt == 0), stop=(mt == 7))
                    b2u = tmpe.tile([128, DIM], f32, tag="b2u")
                    nc.vector.tensor_tensor(b2u[:], psU[:], b2bc[:], AL.mult)
                    for ns in range(4):
                        nc.vector.tensor_add(acc2[:, ns, :], acc2[:, ns, :], b2u[:])

                if DEBUG:
                    nc.sync.dma_start(dbg_w1f[:], w1f[:])
                    nc.sync.dma_start(dbg_u[:], u16[:])
                    nc.sync.dma_start(dbg_acc2[:], acc2[:])

                # ---- Phase F: output projection -----------------------------
                with tc.tile_pool(name="phf", bufs=1) as pf, \
                     tc.tile_pool(name="wos", bufs=2) as wos, \
                     tc.tile_pool(name="psf", bufs=2, space="PSUM") as psf, \
                     tc.tile_pool(name="psft", bufs=2, space="PSUM") as psft:
                    ident = pf.tile([128, 128], f32)
                    make_identity(nc, ident[:])
                    obbc = pf.tile([128, DIM], f32)
                    nc.sync.dma_start(obbc[:], ob[0:1, :].to_broadcast((128, DIM)))
                    wor = wo_T.rearrange("(c p) n -> p c n", p=128)
                    outT = pf.tile([128, 6, NH], f32r)
                    for ns in range(4):
                        for jc in range(6):
                            psT = psft.tile([128, 128], f32, tag="psT")
                            nc.tensor.transpose(psT[:], acc2[:, ns, jc * 128:(jc + 1) * 128],
                                                ident[:])
                            nc.vector.tensor_copy(outT[:, jc, ns * 128:(ns + 1) * 128],
                                                  psT[:])
                    y_sb = pf.tile([128, 4, DIM], f32)
                    for ns in range(4):
                        psY = psf.tile([128, 512], f32, tag="psY")
                        psY2 = psf.tile([128, 256], f32, tag="psY2")
                        for jc in range(6):
                            wo_t = wos.tile([128, DIM], f32r, tag="wo")
                            nc.sync.dma_start(wo_t[:], wor[:, jc, :])
                            nc.tensor.matmul(psY[:], outT[:, jc, ns * 128:(ns + 1) * 128],
                                             wo_t[:, 0:512], start=(jc == 0), stop=(jc == 5))
                            nc.tensor.matmul(psY2[:], outT[:, jc, ns * 128:(ns + 1) * 128],
                                             wo_t[:, 512:768], start=(jc == 0), stop=(jc == 5))
                        nc.vector.tensor_tensor(y_sb[:, ns, 0:512], psY[:],
                                                obbc[:, 0:512], AL.add)
                        nc.vector.tensor_tensor(y_sb[:, ns, 512:768], psY2[:],
                                                obbc[:, 512:768], AL.add)
                    nc.sync.dma_start(y.rearrange("(ns p) j -> p ns j", p=128), y_sb[:])

    nc.compile()
    return nc


def kernel(x, qkv_w, proj_l_w, proj_l_b, proj_w_w, proj_w_b, lamb,
           proj_out_w, proj_out_b):
    x = np.asarray(x, dtype=np.float32)
    qkv_w = np.asarray(qkv_w, dtype=np.float32)
    proj_l_w = np.asarray(proj_l_w, dtype=np.float32)
    proj_l_b = np.asarray(proj_l_b, dtype=np.float32)
    proj_w_w = np.asarray(proj_w_w, dtype=np.float32)
    proj_w_b = np.asarray(proj_w_b, dtype=np.float32)
    lamb = np.asarray(lamb, dtype=np.float32)
    proj_out_w = np.asarray(proj_out_w, dtype=np.float32)
    proj_out_b = np.asarray(proj_out_b, dtype=np.float32)

    nc = _build()

    wq_T = np.ascontiguousarray(qkv_w[:DIM].T) * np.float32(SCALE)
    wk_T = np.ascontiguousarray(qkv_w[DIM:2 * DIM].T)
    wv_T = np.ascontiguousarray(qkv_w[2 * DIM:].T)
    wo_T = np.ascontiguousarray(proj_out_w.T)

    w1v = np.empty((128, 72), dtype=np.float32)
    for g in range(HEADS):
        for i in range(6):
            w1v[:64, g * 6 + i] = proj_l_w[g, 2 * i]
            w1v[64:, g * 6 + i] = proj_l_w[g, 2 * i + 1]
    b1bc = np.tile(proj_l_b[None, :], (128, 1)).astype(np.float32)
    # w2f[0, g*768 + g'*64 + d] = proj_w_w[g', g]
    w2f = np.repeat(proj_w_w.T, HD, axis=1).reshape(1, HEADS * DIM).astype(np.float32)
    uc1 = np.repeat(1.0 - 2.0 * lamb, HD)[None, :].astype(np.float32)
    uc2 = np.repeat(3.0 * lamb, HD)[None, :].astype(np.float32)
    b2blk = np.repeat(proj_w_b, HD)[None, :].astype(np.float32)
    ob = proj_out_b[None, :].astype(np.float32)

    in_maps = []
    for c in range(8):
        b, half = c // 2, c % 2
        in_maps.append({
            "xh_T": np.ascontiguousarray(x[b, half * NH:(half + 1) * NH, :].T),
            "xf_T": np.ascontiguousarray(x[b].T),
            "wq_T": wq_T, "wk_T": wk_T, "wv_T": wv_T, "wo_T": wo_T,
            "w1v": w1v, "b1bc": b1bc, "w2f": w2f,
            "uc1": uc1, "uc2": uc2, "b2blk": b2blk, "ob": ob,
        })

    res = run_bass_kernel_spmd(nc, in_maps, core_ids=list(range(8)),
                               trace=TRACE, **TRACE_KW)
    kernel.last_results = res
    kernel.last_nc = nc
    kernel.last_in_maps = in_maps

    out = np.empty((B, N, DIM), dtype=np.float32)
    for c in range(8):
        b, half = c // 2, c % 2
        out[b, half * NH:(half + 1) * NH, :] = res.results[c]["y"]
    return out
